# revision 8
# baseline (speedup 1.0000x reference)
"""Trainium2 kernel for nn_AsyncNaive (ragged multimodal LSTM + linear head).

Algorithm (validated on host in emulate.py):
  1. The fusion head is linear: out[n,t] = sum_m Wc_m . h_m[n,t] + bc, scaled by
     lstm_masks, where Wc = fuse_W2 @ fuse_W1 (and the reference's no-transpose
     reshape scrambles (n,t) -> we reproduce that on host).
  2. Per modality, only substeps with (t < seq_len) & mask matter; inactive
     substeps are identity. Each sequence's active substeps are packed into a
     dense chain (max ~504 steps instead of 1024), garbage beyond a sequence's
     own length is never read back.
  3. Device: one SPMD program on 3 cores (one per modality, uniform padded
     shapes H->384, 4H->1536, D->301). Phase A projects packed inputs
     (zx = [x;1] @ [Wih;bias], fp16). Phase B runs the sequential LSTM chain:
     gates accumulate in PSUM (zx injected via identity-matmul, recurrent
     matmuls fp16 stationary weights), sigmoid/tanh on ScalarE, elementwise on
     VectorE, per-step output scalar s = Wc.h accumulated on-chip. Only s
     ([504,64] floats per modality) is downloaded.

mc order: [i0 f0 o0 i1 f1 o1 i2 f2 | o2 g0 g1 g2]; psum bankA = mc0..7,
bankB = mc8..11. zx DRAM layout [128, (blk, mc, j, lane)].
"""

import os

import numpy as np

N, T, K = 64, 256, 4
MODS = ["linguistic", "acoustic", "image"]
DIMS = {"linguistic": 300, "acoustic": 88, "image": 128}
HID = {"linguistic": 300, "acoustic": 64, "image": 128}
GIDX = {"i": 0, "f": 1, "g": 2, "o": 3}  # torch gate order in 4H
MC_GATE = [("i", 0), ("f", 0), ("o", 0), ("i", 1), ("f", 1), ("o", 1),
           ("i", 2), ("f", 2), ("o", 2), ("g", 0), ("g", 1), ("g", 2)]
Hp, G4P, B, BLK, NMC = 384, 1536, 64, 8, 12
Dp = 301  # 300 data rows + 1 bias row
DKC = (128, 128, 45)  # Dp chunks
CBLK = NMC * BLK * B  # zx cols per block = 6144
RUNNER = None  # test hook: callable(nc, in_maps) -> list[dict] of outputs
LAST_EXEC_NS = None
LAST_RUN_WALL_S = None


# ---------------------------------------------------------------- device ----

def build_program(nblk):
    import concourse.bass as bass
    import concourse.mybir as mybir
    import concourse.tile as tile

    F32 = mybir.dt.float32
    F16 = mybir.dt.float16
    nc = bass.Bass()

    xpk = nc.declare_dram_parameter("xpk", [Dp, nblk * BLK * B], F16, isOutput=False)
    wih = nc.declare_dram_parameter("wih", [Dp, G4P], F16, isOutput=False)
    whh = nc.declare_dram_parameter("whh", [Hp, G4P], F16, isOutput=False)
    wc = nc.declare_dram_parameter("wc", [128, 3], F16, isOutput=False)
    eye = nc.declare_dram_parameter("eye", [128, 128], F16, isOutput=False)
    s_out = nc.declare_dram_parameter("s_out", [nblk, BLK * B], F32, isOutput=True)

    with tile.TileContext(nc) as tc:
        with (
            tc.tile_pool(name="const", bufs=1) as cpool,
            tc.tile_pool(name="zxdram", bufs=1, space=bass.MemorySpace.DRAM) as dpool,
        ):
            zxd = dpool.tile([128, nblk * CBLK], F16, name="zxd")

            # ---- phase A: projection ----
            wih_t = []
            r0 = 0
            for kc, rows in enumerate(DKC):
                t_ = cpool.tile([rows, G4P], F16, name=f"wih{kc}")
                nc.sync.dma_start(t_[:], wih[r0:r0 + rows, :])
                wih_t.append(t_)
                r0 += rows
            with (
                tc.tile_pool(name="xin", bufs=3) as xpool,
                tc.tile_pool(name="pj", bufs=4, space=bass.MemorySpace.PSUM) as pjpool,
                tc.tile_pool(name="zq", bufs=2) as zqpool,
            ):
                for cc in range(nblk):
                    xt = []
                    r0 = 0
                    for kc, rows in enumerate(DKC):
                        t_ = xpool.tile([rows, 512], F16, name=f"x{kc}")
                        nc.sync.dma_start(
                            t_[:], xpk[r0:r0 + rows, cc * 512:(cc + 1) * 512])
                        xt.append(t_)
                        r0 += rows
                    zq = zqpool.tile([128, CBLK], F16, name="zq")
                    for mc in range(NMC):
                        pt = pjpool.tile([128, 512], F32, name="pt")
                        for kc in range(3):
                            nc.tensor.matmul(
                                pt[:], wih_t[kc][:, mc * 128:(mc + 1) * 128],
                                xt[kc][:], start=(kc == 0), stop=(kc == 2))
                        dst = zq[:, mc * 512:(mc + 1) * 512]
                        if mc % 2 == 0:
                            nc.scalar.copy(dst, pt[:])
                        else:
                            nc.vector.tensor_copy(dst, pt[:])
                    nc.sync.dma_start(
                        zxd[:, cc * CBLK:(cc + 1) * CBLK], zq[:])

            # ---- phase B: recurrence ----
            whh_t = [cpool.tile([128, G4P], F16, name=f"whh{kc}") for kc in range(3)]
            for kc in range(3):
                nc.sync.dma_start(whh_t[kc][:], whh[kc * 128:(kc + 1) * 128, :])
            wc_t = cpool.tile([128, 3], F16, name="wc")
            nc.sync.dma_start(wc_t[:], wc[:, :])
            eye_t = cpool.tile([128, 128], F16, name="eye")
            nc.sync.dma_start(eye_t[:], eye[:, :])

            with (
                tc.tile_pool(name="zxb", bufs=2) as zxpool,
                tc.tile_pool(name="state", bufs=1) as spool,
                tc.tile_pool(name="hpool", bufs=3) as hpool,
                tc.tile_pool(name="work", bufs=3) as wpool,
                tc.tile_pool(name="zp", bufs=2, space=bass.MemorySpace.PSUM) as zppool,
                tc.tile_pool(name="sp", bufs=2, space=bass.MemorySpace.PSUM) as sppool,
                tc.tile_pool(name="so", bufs=2) as sopool,
            ):
                c_t = spool.tile([128, 192], F32, name="c")
                nc.vector.memset(c_t[:], 0.0)
                h_prev = hpool.tile([128, 192], F16, name="h")
                nc.vector.memset(h_prev[:], 0.0)

                for blk in range(nblk):
                    zxb = zxpool.tile([128, CBLK], F16, name="zxb")
                    nc.sync.dma_start(
                        zxb[:], zxd[:, blk * CBLK:(blk + 1) * CBLK])
                    zxv = zxb[:].rearrange("p (m j l) -> p m j l", m=NMC, j=BLK, l=B)
                    s_ps = sppool.tile([1, 512], F32, name="sps")
                    for jj in range(BLK):
                        zpa = zppool.tile([128, 512], F32, name="zpa")
                        zpb = zppool.tile([128, 512], F32, name="zpb")
                        nc.tensor.matmul(
                            zpa[:].rearrange("p (m l) -> p m l", m=8),
                            eye_t[:], zxv[:, 0:8, jj, :],
                            start=True, stop=False, skip_group_check=True)
                        nc.tensor.matmul(
                            zpb[:, 0:256].rearrange("p (m l) -> p m l", m=4),
                            eye_t[:], zxv[:, 8:12, jj, :],
                            start=True, stop=False, skip_group_check=True)
                        for kc in range(3):
                            for mc in range(NMC):
                                dst = zpa[:, mc * 64:(mc + 1) * 64] if mc < 8 else \
                                    zpb[:, (mc - 8) * 64:(mc - 7) * 64]
                                nc.tensor.matmul(
                                    dst, whh_t[kc][:, mc * 128:(mc + 1) * 128],
                                    h_prev[:, kc * 64:(kc + 1) * 64],
                                    start=False, stop=(kc == 2),
                                    skip_group_check=True)
                        zs = wpool.tile([128, 576], F32, name="zs")
                        tg = wpool.tile([128, 192], F32, name="tg")
                        tct = wpool.tile([128, 192], F32, name="tct")
                        h_new = hpool.tile([128, 192], F16, name="h")
                        t1 = wpool.tile([128, 192], F32, name="t1")
                        t2 = wpool.tile([128, 192], F32, name="t2")
                        SIG = mybir.ActivationFunctionType.Sigmoid
                        TANH = mybir.ActivationFunctionType.Tanh
                        nc.scalar.activation(zs[:, 0:512], zpa[:, :], SIG)
                        nc.scalar.activation(zs[:, 512:576], zpb[:, 0:64], SIG)
                        nc.scalar.activation(tg[:, :], zpb[:, 64:256], TANH)
                        zsv = zs[:].rearrange("p (g l) -> p g l", l=64)
                        for (c0, c1) in ((0, 2), (2, 3)):
                            csl = slice(c0 * 64, c1 * 64)
                            i_ap = zsv[:, c0 * 3 + 0:c1 * 3:3, :]
                            f_ap = zsv[:, c0 * 3 + 1:c1 * 3:3, :]
                            o_ap = zsv[:, c0 * 3 + 2:c1 * 3:3, :]
                            gv = tg[:, csl].rearrange("p (g l) -> p g l", l=64)
                            cv = c_t[:, csl].rearrange("p (g l) -> p g l", l=64)
                            t1v = t1[:, csl].rearrange("p (g l) -> p g l", l=64)
                            t2v = t2[:, csl].rearrange("p (g l) -> p g l", l=64)
                            nc.vector.tensor_mul(t2v, f_ap, cv)
                            nc.vector.tensor_mul(t1v, i_ap, gv)
                            nc.vector.tensor_tensor(
                                cv, t1v, t2v, op=mybir.AluOpType.add)
                            nc.scalar.activation(tct[:, csl], c_t[:, csl], TANH)
                            tcv = tct[:, csl].rearrange("p (g l) -> p g l", l=64)
                            hv = h_new[:, csl].rearrange("p (g l) -> p g l", l=64)
                            nc.vector.tensor_mul(hv, o_ap, tcv)
                        for r in range(3):
                            nc.tensor.matmul(
                                s_ps[:, jj * 64:(jj + 1) * 64],
                                wc_t[:, r:r + 1], h_new[:, r * 64:(r + 1) * 64],
                                start=(r == 0), stop=(r == 2),
                                skip_group_check=True)
                        h_prev = h_new
                    s_sb = sopool.tile([1, 512], F32, name="ssb")
                    nc.vector.tensor_copy(s_sb[:], s_ps[:])
                    nc.sync.dma_start(s_out[blk:blk + 1, :], s_sb[:])
    return nc


# ------------------------------------------------------------------ host ----

def _pack_indices(mask, seq_len):
    """active flags [N, T*K] -> (idx [N, L] padded substep indices, cum [N, T])."""
    act = mask & (np.arange(T)[None, :, None] < seq_len[:, None, None])
    flat = act.reshape(N, T * K)
    L = int(flat.sum(1).max())
    idx = np.zeros((N, L), np.int64)
    for n in range(N):
        w = np.nonzero(flat[n])[0]
        if len(w):
            idx[n, :len(w)] = w
            idx[n, len(w):] = w[-1]
    cum = act.reshape(N, T, K).sum(2).cumsum(1)
    return idx, cum


def _col_map(H):
    """dest padded col (1536) -> src col in [4H] or -1."""
    dest = np.full(G4P, -1, np.int64)
    for mc, (g, r) in enumerate(MC_GATE):
        p = np.arange(128)
        rows = r * 128 + p
        ok = rows < H
        dest[mc * 128 + p[ok]] = GIDX[g] * H + rows[ok]
    return dest


def _prep_core_inputs(m, inputs, idx, lmax):
    D, H = DIMS[m], HID[m]
    nblk = lmax // BLK
    x = np.asarray(inputs[f"x_{m}"], np.float32).reshape(N, T * K, D)
    xg = np.take_along_axis(x, idx[:, :, None], axis=1)  # [N, L, D]
    xpk = np.zeros((Dp, nblk * BLK * B), np.float16)
    xpk[:D, :] = xg.transpose(2, 1, 0).reshape(D, lmax * B)
    xpk[D, :] = 1.0

    cmap = _col_map(H)
    sel = cmap >= 0
    wihT = np.asarray(inputs[f"Wih_{m}"], np.float32).T  # [D, 4H]
    bias = (np.asarray(inputs[f"bih_{m}"], np.float32)
            + np.asarray(inputs[f"bhh_{m}"], np.float32))
    wih_p = np.zeros((Dp, G4P), np.float16)
    wih_p[:D, sel] = wihT[:, cmap[sel]].astype(np.float16)
    wih_p[D, sel] = bias[cmap[sel]].astype(np.float16)

    whhT = np.asarray(inputs[f"Whh_{m}"], np.float32).T  # [H, 4H]
    whh_p = np.zeros((Hp, G4P), np.float16)
    whh_p[:H, sel] = whhT[:, cmap[sel]].astype(np.float16)
    return {"xpk": xpk, "wih": wih_p, "whh": whh_p}


def _device_path(inputs, seq_len):
    from concourse.bass_utils import run_bass_kernel_spmd

    W2 = np.asarray(inputs["fuse_W2"], np.float32)
    W1 = np.asarray(inputs["fuse_W1"], np.float32)
    Wc = (W2 @ W1)[0]  # [492]
    col0 = {"linguistic": 0, "acoustic": 300, "image": 364}

    packs = {}
    lmax = 0
    for m in MODS:
        mask = np.asarray(inputs[f"mask_{m}"]).astype(bool)
        idx, cum = _pack_indices(mask, seq_len)
        packs[m] = (idx, cum)
        lmax = max(lmax, idx.shape[1])
    lmax = ((lmax + BLK - 1) // BLK) * BLK
    nblk = lmax // BLK

    eye = np.eye(128, dtype=np.float16)
    in_maps = []
    for m in MODS:
        idx, _ = packs[m]
        idx_pad = np.concatenate(
            [idx, np.repeat(idx[:, -1:], lmax - idx.shape[1], 1)], axis=1)
        packs[m] = (idx_pad, packs[m][1])
        im = _prep_core_inputs(m, inputs, idx_pad, lmax)
        H = HID[m]
        wc_t = np.zeros((128, 3), np.float16)
        wcm = Wc[col0[m]:col0[m] + H]
        for r in range(3):
            lo, hi = r * 128, min((r + 1) * 128, H)
            if lo < H:
                wc_t[:hi - lo, r] = wcm[lo:hi].astype(np.float16)
        im["wc"] = wc_t
        im["eye"] = eye
        in_maps.append(im)

    nc = build_program(nblk)
    if RUNNER is not None:
        results = RUNNER(nc, in_maps)
    else:
        import time
        global LAST_EXEC_NS, LAST_RUN_WALL_S
        t0 = time.time()
        res = run_bass_kernel_spmd(nc, in_maps, core_ids=[0, 1, 2])
        LAST_RUN_WALL_S = time.time() - t0
        LAST_EXEC_NS = res.exec_time_ns
        results = res.results

    s_total = None
    for ci, m in enumerate(MODS):
        s_dev = np.asarray(results[ci]["s_out"], np.float32)
        s_packed = s_dev.reshape(lmax, B).T  # [n, j]
        _, cum = packs[m]
        j = np.maximum(cum - 1, 0)
        valid = (cum > 0) & (np.arange(T)[None, :] < seq_len[:, None])
        gat = np.take_along_axis(s_packed, j, axis=1)
        s_m = np.where(valid, gat, 0.0)
        # reference reshape quirk: [T,N,H] -> [N,T,H] with no transpose
        s_m = s_m.T.reshape(N, T)
        s_total = s_m if s_total is None else s_total + s_m

    b1 = np.asarray(inputs["fuse_b1"], np.float32)
    b2 = np.asarray(inputs["fuse_b2"], np.float32)
    bc = float((W2 @ b1 + b2).reshape(-1)[0])
    out = (s_total + bc)[:, :, None] * np.asarray(inputs["lstm_masks"], np.float32)
    return out.astype(np.float32)


# ------------------------------------------------- host fallback (exact) ----

def _sigmoid(v):
    return 1.0 / (1.0 + np.exp(-v))


def _host_path(inputs, seq_len):
    W2 = np.asarray(inputs["fuse_W2"], np.float32)
    W1 = np.asarray(inputs["fuse_W1"], np.float32)
    Wc = (W2 @ W1)[0]
    col0 = {"linguistic": 0, "acoustic": 300, "image": 364}
    s_total = None
    for m in MODS:
        D, H = DIMS[m], HID[m]
        mask = np.asarray(inputs[f"mask_{m}"]).astype(bool)
        idx, cum = _pack_indices(mask, seq_len)
        L = idx.shape[1]
        x = np.asarray(inputs[f"x_{m}"], np.float32).reshape(N, T * K, D)
        Wih = np.asarray(inputs[f"Wih_{m}"], np.float32)
        bias = (np.asarray(inputs[f"bih_{m}"], np.float32)
                + np.asarray(inputs[f"bhh_{m}"], np.float32))
        zx = np.take_along_axis(x, idx[:, :, None], 1) @ Wih.T + bias
        WhhT = np.asarray(inputs[f"Whh_{m}"], np.float32).T
        Wcm = Wc[col0[m]:col0[m] + H]
        h = np.zeros((N, H), np.float32)
        c = np.zeros((N, H), np.float32)
        s_packed = np.zeros((N, L), np.float32)
        for j in range(L):
            z = zx[:, j] + h @ WhhT
            i = _sigmoid(z[:, :H]); f = _sigmoid(z[:, H:2 * H])
            g = np.tanh(z[:, 2 * H:3 * H]); o = _sigmoid(z[:, 3 * H:])
            c = f * c + i * g
            h = o * np.tanh(c)
            s_packed[:, j] = h @ Wcm
        jv = np.maximum(cum - 1, 0)
        valid = (cum > 0) & (np.arange(T)[None, :] < seq_len[:, None])
        s_m = np.where(valid, np.take_along_axis(s_packed, jv, 1), 0.0)
        s_m = s_m.T.reshape(N, T)
        s_total = s_m if s_total is None else s_total + s_m
    b1 = np.asarray(inputs["fuse_b1"], np.float32)
    b2 = np.asarray(inputs["fuse_b2"], np.float32)
    bc = float((W2 @ b1 + b2).reshape(-1)[0])
    out = (s_total + bc)[:, :, None] * np.asarray(inputs["lstm_masks"], np.float32)
    return out.astype(np.float32)


def kernel(**inputs):
    seq_len = np.asarray(inputs["seq_length"]).astype(np.int64)
    if os.environ.get("KERNEL_FORCE_HOST"):
        return _host_path(inputs, seq_len)
    try:
        return _device_path(inputs, seq_len)
    except Exception:
        import traceback
        traceback.print_exc()
        print("[kernel] device path failed; host fallback")
        return _host_path(inputs, seq_len)


# revision 19
# speedup vs baseline: 1.1546x; 1.1546x over previous
"""Trainium2 kernel for nn_AsyncNaive (ragged multimodal LSTM + linear head).

Algorithm (validated on host in emulate.py):
  1. The fusion head is linear: out[n,t] = sum_m Wc_m . h_m[n,t] + bc, scaled by
     lstm_masks, where Wc = fuse_W2 @ fuse_W1 (and the reference's no-transpose
     reshape scrambles (n,t) -> we reproduce that on host).
  2. Per modality, only substeps with (t < seq_len) & mask matter; inactive
     substeps are identity. Each sequence's active substeps are packed into a
     dense chain (max ~504 steps instead of 1024), garbage beyond a sequence's
     own length is never read back.
  3. Device: one SPMD program on 3 cores (one per modality, uniform padded
     shapes H->384, 4H->1536, D->301). Phase A projects packed inputs
     (zx = [x;1] @ [Wih;bias], fp16). Phase B runs the sequential LSTM chain:
     gates accumulate in PSUM (zx injected via identity-matmul, recurrent
     matmuls fp16 stationary weights), sigmoid/tanh on ScalarE, elementwise on
     VectorE, per-step output scalar s = Wc.h accumulated on-chip. Only s
     ([504,64] floats per modality) is downloaded.

mc order: [i0 f0 o0 i1 f1 o1 i2 f2 | o2 g0 g1 g2]; psum bankA = mc0..7,
bankB = mc8..11. zx DRAM layout [128, (blk, mc, j, lane)].
"""

import os

import numpy as np

N, T, K = 64, 256, 4
MODS = ["linguistic", "acoustic", "image"]
DIMS = {"linguistic": 300, "acoustic": 88, "image": 128}
HID = {"linguistic": 300, "acoustic": 64, "image": 128}
GIDX = {"i": 0, "f": 1, "g": 2, "o": 3}  # torch gate order in 4H
MC_GATE = [("i", 0), ("f", 0), ("o", 0), ("i", 1), ("f", 1), ("o", 1),
           ("i", 2), ("f", 2), ("o", 2), ("g", 0), ("g", 1), ("g", 2)]
Hp, G4P, B, BLK, NMC = 384, 1536, 64, 8, 12
Dp = 301  # 300 data rows + 1 bias row
DKC = (128, 128, 45)  # Dp chunks
CBLK = NMC * BLK * B  # zx cols per block = 6144
RUNNER = None  # test hook: callable(nc, in_maps) -> list[dict] of outputs
LAST_EXEC_NS = None
LAST_RUN_WALL_S = None


# ---------------------------------------------------------------- device ----

def build_program(nblk):
    import concourse.bass as bass
    import concourse.mybir as mybir
    import concourse.tile as tile
    from concourse.bacc import Bacc

    F32 = mybir.dt.float32
    F16 = mybir.dt.float16
    nc = Bacc()

    xpk = nc.declare_dram_parameter("xpk", [Dp, nblk * BLK * B], F16, isOutput=False)
    wih = nc.declare_dram_parameter("wih", [Dp, G4P], F16, isOutput=False)
    whh = nc.declare_dram_parameter("whh", [Hp, G4P], F16, isOutput=False)
    wc = nc.declare_dram_parameter("wc", [128, 45], F16, isOutput=False)
    eye = nc.declare_dram_parameter("eye", [128, 128], F16, isOutput=False)
    s_out = nc.declare_dram_parameter("s_out", [BLK, nblk * B], F32, isOutput=True)

    with tile.TileContext(nc) as tc:
        with (
            tc.tile_pool(name="const", bufs=1) as cpool,
            tc.tile_pool(name="zxdram", bufs=1, space=bass.MemorySpace.DRAM) as dpool,
        ):
            zxd = dpool.tile([128, nblk * CBLK], F16, name="zxd")

            # ---- phase A: projection ----
            wih_t = []
            r0 = 0
            for kc, rows in enumerate(DKC):
                t_ = cpool.tile([rows, G4P], F16, name=f"wih{kc}")
                nc.sync.dma_start(t_[:], wih[r0:r0 + rows, :])
                wih_t.append(t_)
                r0 += rows
            with (
                tc.tile_pool(name="xin", bufs=3) as xpool,
                tc.tile_pool(name="pj", bufs=4, space=bass.MemorySpace.PSUM) as pjpool,
                tc.tile_pool(name="zq", bufs=2) as zqpool,
            ):
                for cc in range(nblk):
                    xt = []
                    r0 = 0
                    for kc, rows in enumerate(DKC):
                        t_ = xpool.tile([rows, 512], F16, name=f"x{kc}")
                        nc.sync.dma_start(
                            t_[:], xpk[r0:r0 + rows, cc * 512:(cc + 1) * 512])
                        xt.append(t_)
                        r0 += rows
                    zq = zqpool.tile([128, CBLK], F16, name="zq")
                    for mc in range(NMC):
                        pt = pjpool.tile([128, 512], F32, name="pt")
                        for kc in range(3):
                            nc.tensor.matmul(
                                pt[:], wih_t[kc][:, mc * 128:(mc + 1) * 128],
                                xt[kc][:], start=(kc == 0), stop=(kc == 2))
                        dst = zq[:, mc * 512:(mc + 1) * 512]
                        nc.scalar.copy(dst, pt[:])
                    # ACT produced zq and issues the out-DMA: RAW is program
                    # order, so the DMA instruction needs no sem waits
                    nc.scalar.dma_start(
                        zxd[:, cc * CBLK:(cc + 1) * CBLK], zq[:])

            # ---- phase B: recurrence ----
            tc.strict_bb_all_engine_barrier()
            whh_t = [cpool.tile([128, G4P], F16, name=f"whh{kc}") for kc in range(3)]
            for kc in range(3):
                nc.sync.dma_start(whh_t[kc][:], whh[kc * 128:(kc + 1) * 128, :])
            wc_t = cpool.tile([128, 45], F16, name="wc")
            nc.sync.dma_start(wc_t[:], wc[:, :])
            eye_t = cpool.tile([128, 128], F16, name="eye")
            nc.sync.dma_start(eye_t[:], eye[:, :])

            with (
                tc.tile_pool(name="zxb", bufs=2) as zxpool,
                tc.tile_pool(name="state", bufs=1) as spool,
                tc.tile_pool(name="hpool", bufs=3) as hpool,
                tc.tile_pool(name="work", bufs=3) as wpool,
                tc.tile_pool(name="zp", bufs=2, space=bass.MemorySpace.PSUM) as zppool,
                tc.tile_pool(name="sp", bufs=2, space=bass.MemorySpace.PSUM) as sppool,
                tc.tile_pool(name="so", bufs=1) as sopool,
            ):
                s_all = sopool.tile([BLK, nblk * B], F32, name="s_all")
                c_t = spool.tile([128, 192], F32, name="c")
                nc.vector.memset(c_t[:], 0.0)
                h_prev = hpool.tile([128, 192], F16, name="h")
                nc.vector.memset(h_prev[:], 0.0)

                for blk in range(nblk):
                    zxb = zxpool.tile([128, CBLK], F16, name="zxb")
                    nc.scalar.dma_start(
                        zxb[:], zxd[:, blk * CBLK:(blk + 1) * CBLK])
                    zxv = zxb[:].rearrange("p (m j l) -> p m j l", m=NMC, j=BLK, l=B)
                    s_ps = sppool.tile([8, 64], F32, name="sps")
                    for jj in range(BLK):
                        zpa = zppool.tile([128, 512], F32, name="zpa")
                        zpb = zppool.tile([128, 512], F32, name="zpb")
                        nc.tensor.matmul(
                            zpa[:].rearrange("p (m l) -> p m l", m=8),
                            eye_t[:], zxv[:, 0:8, jj, :],
                            start=True, stop=False, skip_group_check=True)
                        nc.tensor.matmul(
                            zpb[:, 0:256].rearrange("p (m l) -> p m l", m=4),
                            eye_t[:], zxv[:, 8:12, jj, :],
                            start=True, stop=False, skip_group_check=True)
                        for kc in range(3):
                            for mc in range(NMC):
                                dst = zpa[:, mc * 64:(mc + 1) * 64] if mc < 8 else \
                                    zpb[:, (mc - 8) * 64:(mc - 7) * 64]
                                nc.tensor.matmul(
                                    dst, whh_t[kc][:, mc * 128:(mc + 1) * 128],
                                    h_prev[:, kc * 64:(kc + 1) * 64],
                                    start=False, stop=(kc == 2),
                                    skip_group_check=True)
                        zs = wpool.tile([128, 576], F32, name="zs")
                        tg = wpool.tile([128, 192], F32, name="tg")
                        tct = wpool.tile([128, 192], F32, name="tct")
                        h_new = hpool.tile([128, 192], F16, name="h")
                        t1 = wpool.tile([128, 192], F32, name="t1")
                        t2 = wpool.tile([128, 192], F32, name="t2")
                        SIG = mybir.ActivationFunctionType.Sigmoid
                        TANH = mybir.ActivationFunctionType.Tanh
                        nc.scalar.activation(zs[:, 0:512], zpa[:, :], SIG)
                        nc.scalar.activation(zs[:, 512:576], zpb[:, 0:64], SIG)
                        nc.scalar.activation(tg[:, :], zpb[:, 64:256], TANH)
                        zsv = zs[:].rearrange("p (g l) -> p g l", l=64)
                        for (c0, c1) in ((0, 2), (2, 3)):
                            csl = slice(c0 * 64, c1 * 64)
                            i_ap = zsv[:, c0 * 3 + 0:c1 * 3:3, :]
                            f_ap = zsv[:, c0 * 3 + 1:c1 * 3:3, :]
                            o_ap = zsv[:, c0 * 3 + 2:c1 * 3:3, :]
                            gv = tg[:, csl].rearrange("p (g l) -> p g l", l=64)
                            cv = c_t[:, csl].rearrange("p (g l) -> p g l", l=64)
                            t1v = t1[:, csl].rearrange("p (g l) -> p g l", l=64)
                            t2v = t2[:, csl].rearrange("p (g l) -> p g l", l=64)
                            nc.vector.tensor_mul(t2v, f_ap, cv)
                            nc.vector.tensor_mul(t1v, i_ap, gv)
                            nc.vector.tensor_tensor(
                                cv, t1v, t2v, op=mybir.AluOpType.add)
                            nc.scalar.activation(tct[:, csl], c_t[:, csl], TANH)
                            tcv = tct[:, csl].rearrange("p (g l) -> p g l", l=64)
                            hv = h_new[:, csl].rearrange("p (g l) -> p g l", l=64)
                            nc.vector.tensor_mul(hv, o_ap, tcv)
                        for r in range(3):
                            nc.tensor.matmul(
                                s_ps[:, :],
                                wc_t[:, r * 15 + 7 - jj:r * 15 + 15 - jj],
                                h_new[:, r * 64:(r + 1) * 64],
                                start=(jj == 0 and r == 0),
                                stop=(jj == BLK - 1 and r == 2),
                                skip_group_check=True)
                        h_prev = h_new
                    nc.vector.tensor_copy(
                        s_all[:, blk * B:(blk + 1) * B], s_ps[:])
                nc.sync.dma_start(s_out[:, :], s_all[:])
    if not nc.is_finalized():
        nc.finalize()
    return nc


# ------------------------------------------------------------------ host ----

def _pack_indices(mask, seq_len):
    """active flags [N, T*K] -> (idx [N, L] padded substep indices, cum [N, T])."""
    act = mask & (np.arange(T)[None, :, None] < seq_len[:, None, None])
    flat = act.reshape(N, T * K)
    L = int(flat.sum(1).max())
    idx = np.zeros((N, L), np.int64)
    for n in range(N):
        w = np.nonzero(flat[n])[0]
        if len(w):
            idx[n, :len(w)] = w
            idx[n, len(w):] = w[-1]
    cum = act.reshape(N, T, K).sum(2).cumsum(1)
    return idx, cum


def _col_map(H):
    """dest padded col (1536) -> src col in [4H] or -1."""
    dest = np.full(G4P, -1, np.int64)
    for mc, (g, r) in enumerate(MC_GATE):
        p = np.arange(128)
        rows = r * 128 + p
        ok = rows < H
        dest[mc * 128 + p[ok]] = GIDX[g] * H + rows[ok]
    return dest


def _prep_core_inputs(m, inputs, idx, lmax):
    D, H = DIMS[m], HID[m]
    nblk = lmax // BLK
    x = np.asarray(inputs[f"x_{m}"], np.float32).reshape(N, T * K, D)
    xg = np.take_along_axis(x, idx[:, :, None], axis=1)  # [N, L, D]
    xpk = np.zeros((Dp, nblk * BLK * B), np.float16)
    xpk[:D, :] = xg.transpose(2, 1, 0).reshape(D, lmax * B)
    xpk[D, :] = 1.0

    cmap = _col_map(H)
    sel = cmap >= 0
    wihT = np.asarray(inputs[f"Wih_{m}"], np.float32).T  # [D, 4H]
    bias = (np.asarray(inputs[f"bih_{m}"], np.float32)
            + np.asarray(inputs[f"bhh_{m}"], np.float32))
    wih_p = np.zeros((Dp, G4P), np.float16)
    wih_p[:D, sel] = wihT[:, cmap[sel]].astype(np.float16)
    wih_p[D, sel] = bias[cmap[sel]].astype(np.float16)

    whhT = np.asarray(inputs[f"Whh_{m}"], np.float32).T  # [H, 4H]
    whh_p = np.zeros((Hp, G4P), np.float16)
    whh_p[:H, sel] = whhT[:, cmap[sel]].astype(np.float16)
    return {"xpk": xpk, "wih": wih_p, "whh": whh_p}


def _device_path(inputs, seq_len):
    from concourse.bass_utils import run_bass_kernel_spmd

    W2 = np.asarray(inputs["fuse_W2"], np.float32)
    W1 = np.asarray(inputs["fuse_W1"], np.float32)
    Wc = (W2 @ W1)[0]  # [492]
    col0 = {"linguistic": 0, "acoustic": 300, "image": 364}

    packs = {}
    lmax = 0
    for m in MODS:
        mask = np.asarray(inputs[f"mask_{m}"]).astype(bool)
        idx, cum = _pack_indices(mask, seq_len)
        packs[m] = (idx, cum)
        lmax = max(lmax, idx.shape[1])
    lmax = ((lmax + BLK - 1) // BLK) * BLK
    nblk = lmax // BLK

    eye = np.eye(128, dtype=np.float16)
    in_maps = []
    for m in MODS:
        idx, _ = packs[m]
        idx_pad = np.concatenate(
            [idx, np.repeat(idx[:, -1:], lmax - idx.shape[1], 1)], axis=1)
        packs[m] = (idx_pad, packs[m][1])
        im = _prep_core_inputs(m, inputs, idx_pad, lmax)
        H = HID[m]
        # shifted-window Wc: segment r is [0]*7 + [wc chunk r] + [0]*7 so the
        # lhsT slice [r*15+7-jj : r*15+15-jj] puts the dot on psum partition jj
        wc_t = np.zeros((128, 45), np.float16)
        wcm = Wc[col0[m]:col0[m] + H]
        for r in range(3):
            lo, hi = r * 128, min((r + 1) * 128, H)
            if lo < H:
                wc_t[:hi - lo, r * 15 + 7] = wcm[lo:hi].astype(np.float16)
        im["wc"] = wc_t
        im["eye"] = eye
        in_maps.append(im)

    nc = build_program(nblk)
    if RUNNER is not None:
        results = RUNNER(nc, in_maps)
    else:
        import time
        global LAST_EXEC_NS, LAST_RUN_WALL_S
        t0 = time.time()
        res = run_bass_kernel_spmd(nc, in_maps, core_ids=[0, 1, 2])
        LAST_RUN_WALL_S = time.time() - t0
        LAST_EXEC_NS = res.exec_time_ns
        results = res.results

    s_total = None
    for ci, m in enumerate(MODS):
        s_dev = np.asarray(results[ci]["s_out"], np.float32)
        # s_out[jj, blk*64+lane] = s(step blk*8+jj, lane)
        s_packed = s_dev.reshape(BLK, lmax // BLK, B).transpose(1, 0, 2) \
            .reshape(lmax, B).T  # [n, j]
        _, cum = packs[m]
        j = np.maximum(cum - 1, 0)
        valid = (cum > 0) & (np.arange(T)[None, :] < seq_len[:, None])
        gat = np.take_along_axis(s_packed, j, axis=1)
        s_m = np.where(valid, gat, 0.0)
        # reference reshape quirk: [T,N,H] -> [N,T,H] with no transpose
        s_m = s_m.T.reshape(N, T)
        s_total = s_m if s_total is None else s_total + s_m

    b1 = np.asarray(inputs["fuse_b1"], np.float32)
    b2 = np.asarray(inputs["fuse_b2"], np.float32)
    bc = float((W2 @ b1 + b2).reshape(-1)[0])
    out = (s_total + bc)[:, :, None] * np.asarray(inputs["lstm_masks"], np.float32)
    return out.astype(np.float32)


# ------------------------------------------------- host fallback (exact) ----

def _sigmoid(v):
    return 1.0 / (1.0 + np.exp(-v))


def _host_path(inputs, seq_len):
    W2 = np.asarray(inputs["fuse_W2"], np.float32)
    W1 = np.asarray(inputs["fuse_W1"], np.float32)
    Wc = (W2 @ W1)[0]
    col0 = {"linguistic": 0, "acoustic": 300, "image": 364}
    s_total = None
    for m in MODS:
        D, H = DIMS[m], HID[m]
        mask = np.asarray(inputs[f"mask_{m}"]).astype(bool)
        idx, cum = _pack_indices(mask, seq_len)
        L = idx.shape[1]
        x = np.asarray(inputs[f"x_{m}"], np.float32).reshape(N, T * K, D)
        Wih = np.asarray(inputs[f"Wih_{m}"], np.float32)
        bias = (np.asarray(inputs[f"bih_{m}"], np.float32)
                + np.asarray(inputs[f"bhh_{m}"], np.float32))
        zx = np.take_along_axis(x, idx[:, :, None], 1) @ Wih.T + bias
        WhhT = np.asarray(inputs[f"Whh_{m}"], np.float32).T
        Wcm = Wc[col0[m]:col0[m] + H]
        h = np.zeros((N, H), np.float32)
        c = np.zeros((N, H), np.float32)
        s_packed = np.zeros((N, L), np.float32)
        for j in range(L):
            z = zx[:, j] + h @ WhhT
            i = _sigmoid(z[:, :H]); f = _sigmoid(z[:, H:2 * H])
            g = np.tanh(z[:, 2 * H:3 * H]); o = _sigmoid(z[:, 3 * H:])
            c = f * c + i * g
            h = o * np.tanh(c)
            s_packed[:, j] = h @ Wcm
        jv = np.maximum(cum - 1, 0)
        valid = (cum > 0) & (np.arange(T)[None, :] < seq_len[:, None])
        s_m = np.where(valid, np.take_along_axis(s_packed, jv, 1), 0.0)
        s_m = s_m.T.reshape(N, T)
        s_total = s_m if s_total is None else s_total + s_m
    b1 = np.asarray(inputs["fuse_b1"], np.float32)
    b2 = np.asarray(inputs["fuse_b2"], np.float32)
    bc = float((W2 @ b1 + b2).reshape(-1)[0])
    out = (s_total + bc)[:, :, None] * np.asarray(inputs["lstm_masks"], np.float32)
    return out.astype(np.float32)


def kernel(**inputs):
    seq_len = np.asarray(inputs["seq_length"]).astype(np.int64)
    if os.environ.get("KERNEL_FORCE_HOST"):
        return _host_path(inputs, seq_len)
    try:
        return _device_path(inputs, seq_len)
    except Exception:
        import traceback
        traceback.print_exc()
        print("[kernel] device path failed; host fallback")
        return _host_path(inputs, seq_len)
